# revision 1
# baseline (speedup 1.0000x reference)
"""ConvSA kernel for Trainium2 (8 NeuronCores, data-parallel over batch).

Computes, per batch element b (one per core):
    q/k/v = conv3x3(feat, W{q,k,v}) + b{q,k,v}        # 256 -> 512 ch, SAME pad
    att   = softmax_j(q^T k);  out = v @ att^T + v    # N = 48*48 = 2304

Strategy: all matmuls in float32r (full-rate fp32 storage, ~13-bit
mantissa inputs). Convs as 18 accumulated matmuls (2 c-chunks x 9 taps)
over a zero-padded [128, 2, 50, 50] SBUF image. Attention computed in the
s^T[j, i] orientation (both QK operands in natural conv-output layout),
with a single global shift constant C (column max of the first 128 i's)
instead of per-row max -- mathematically identical softmax, safe in fp32.
p = exp(s - C) stays unnormalized; rowsums via ones-vector matmul;
normalization folded into the output epilogue.
"""
import numpy as np
from contextlib import ExitStack

import concourse.bass as bass
import concourse.tile as tile
from concourse import bacc, bass_utils, mybir
from concourse.masks import make_identity

F32 = mybir.dt.float32
F32R = mybir.dt.float32r

B, C, H, W = 8, 256, 48, 48
E = 512
N = H * W            # 2304
CC = C // 128        # 2 c-chunks
OC = E // 128        # 4 o-chunks / e-chunks
JC = N // 128        # 18 j-chunks
NT = [(0, 10), (10, 10), (20, 10), (30, 10), (40, 8)]     # conv row tiles
IT = [(0, 512), (512, 512), (1024, 512), (1536, 512), (2048, 256)]  # i tiles

_CACHE = {}


def _build():
    nc = bacc.Bacc("TRN2", target_bir_lowering=False, debug=False, num_devices=B)

    xp_ap = nc.dram_tensor("xpad", [128, CC, 2500], F32R, kind="ExternalInput").ap()
    w_aps = {
        cn: nc.dram_tensor(f"w{cn}", [OC, 128, CC, 9, 128], F32R, kind="ExternalInput").ap()
        for cn in "qkv"
    }
    b_aps = {
        cn: nc.dram_tensor(f"b{cn}", [128, OC], F32, kind="ExternalInput").ap()
        for cn in "qkv"
    }
    out_ap = nc.dram_tensor("out", [OC, 128, N], F32, kind="ExternalOutput").ap()

    with tile.TileContext(nc) as tc, ExitStack() as ctx:
        res = ctx.enter_context(tc.tile_pool(name="res", bufs=1))
        k_res = res.tile([128, OC, N], F32R, tag="k")
        q_res = res.tile([128, OC, N], F32R, tag="q")
        vT = res.tile([128, JC, E], F32R, tag="vT")
        bias_t = {cn: res.tile([128, OC], F32, tag=f"b{cn}", name=f"bias_{cn}")
                  for cn in "qkv"}
        ones_col = res.tile([128, 1], F32R, tag="oc")
        ones_row = res.tile([1, 128], F32R, tag="or")
        negC = res.tile([128, 1], F32, tag="negc")
        ident = res.tile([128, 128], F32R, tag="id")

        dram = ctx.enter_context(tc.tile_pool(name="dram", bufs=1, space="DRAM"))
        v_scr = dram.tile([OC, 128, N], F32R)

        for cn in "qkv":
            nc.sync.dma_start(out=bias_t[cn], in_=b_aps[cn])

        # ---------------- conv phase ----------------
        with tc.tile_pool(name="xw", bufs=1) as xwp, \
             tc.tile_pool(name="w", bufs=3) as wp, \
             tc.tile_pool(name="vst", bufs=2) as vstp, \
             tc.tile_pool(name="cps", bufs=2, space="PSUM") as cps:
            ident_raw = xwp.tile([128, 128], F32, tag="idr")
            make_identity(nc, ident_raw)
            nc.vector.tensor_copy(out=ident, in_=ident_raw)
            ones_raw = xwp.tile([128, 1], F32, tag="onr")
            nc.vector.memset(ones_raw, 1.0)
            nc.vector.tensor_copy(out=ones_col, in_=ones_raw)
            ones_raw2 = xwp.tile([1, 128], F32, tag="onr2")
            nc.vector.memset(ones_raw2, 1.0)
            nc.vector.tensor_copy(out=ones_row, in_=ones_raw2)
            xpad_t = xwp.tile([128, CC, 50, 50], F32R, tag="x")
            nc.sync.dma_start(
                out=xpad_t.rearrange("p c h w -> p c (h w)"), in_=xp_ap
            )

            def conv(cn, sink):
                for oc in range(OC):
                    w_t = wp.tile([128, CC, 9, 128], F32R, tag="w")
                    nc.sync.dma_start(out=w_t, in_=w_aps[cn][oc])
                    for (y0, rr) in NT:
                        ps = cps.tile([128, rr * 48], F32, tag="conv")
                        first = True
                        for cc in range(CC):
                            for ky in range(3):
                                for kx in range(3):
                                    rhs = xpad_t[:, cc, y0 + ky:y0 + ky + rr, kx:kx + 48]
                                    nc.tensor.matmul(
                                        ps, w_t[:, cc, ky * 3 + kx, :], rhs,
                                        start=first, stop=(cc == CC - 1 and ky == 2 and kx == 2),
                                    )
                                    first = False
                        sink(cn, oc, y0, rr, ps)

            def to_res(dst):
                def sink(cn, oc, y0, rr, ps):
                    nc.scalar.activation(
                        out=dst[:, oc, y0 * 48:(y0 + rr) * 48], in_=ps,
                        func=mybir.ActivationFunctionType.Identity,
                        bias=bias_t[cn][:, oc:oc + 1], scale=1.0,
                    )
                return sink

            conv("k", to_res(k_res))
            conv("q", to_res(q_res))

            # ---- global shift constant C (hidden under V conv) ----
            # C = max over i in [0,256) x j in [0,1024) of s -- any constant
            # with  rowmax-80 <= C <= globalmax+88  keeps exp() in fp32 range,
            # and softmax is shift-invariant so the result is exact.
            with tc.tile_pool(name="mps", bufs=1, space="PSUM") as mps, \
                 tc.tile_pool(name="msb", bufs=1) as msb, \
                 tc.tile_pool(name="nps", bufs=1, space="PSUM") as nps:
                mini = mps.tile([128, 8, 256], F32)
                for jc in range(8):
                    for ec in range(OC):
                        nc.tensor.matmul(
                            mini[:, jc, :], k_res[:, ec, jc * 128:(jc + 1) * 128],
                            q_res[:, ec, 0:256], start=(ec == 0), stop=(ec == OC - 1),
                        )
                m1 = msb.tile([128, 1], F32R, tag="m1")
                nc.vector.reduce_max(out=m1, in_=mini, axis=mybir.AxisListType.XY)
                tpm = nps.tile([1, 128], F32R, tag="tp")
                nc.tensor.transpose(tpm, m1, ident)
                cneg = msb.tile([1, 2], F32R, tag="cn")
                nc.vector.reduce_max(out=cneg[:, 0:1], in_=tpm,
                                     axis=mybir.AxisListType.X, negate=True)
                nc.vector.tensor_copy(out=cneg[:, 1:2], in_=cneg[:, 0:1])
                ncps = nps.tile([128, 2], F32, tag="ncps")
                nc.tensor.matmul(ncps, ones_row, cneg, start=True, stop=True)
                nc.vector.tensor_copy(out=negC, in_=ncps[:, 0:1])

            # v conv: stage per o-chunk, DMA to scratch + transpose into vT
            with tc.tile_pool(name="tps", bufs=2, space="PSUM") as tps:
                for oc in range(OC):
                    w_t = wp.tile([128, CC, 9, 128], F32R, tag="w")
                    nc.sync.dma_start(out=w_t, in_=w_aps["v"][oc])
                    vs = vstp.tile([128, N], F32R, tag="vs")
                    for (y0, rr) in NT:
                        ps = cps.tile([128, rr * 48], F32, tag="conv")
                        first = True
                        for cc in range(CC):
                            for ky in range(3):
                                for kx in range(3):
                                    rhs = xpad_t[:, cc, y0 + ky:y0 + ky + rr, kx:kx + 48]
                                    nc.tensor.matmul(
                                        ps, w_t[:, cc, ky * 3 + kx, :], rhs,
                                        start=first, stop=(cc == CC - 1 and ky == 2 and kx == 2),
                                    )
                                    first = False
                        nc.scalar.activation(
                            out=vs[:, y0 * 48:(y0 + rr) * 48], in_=ps,
                            func=mybir.ActivationFunctionType.Identity,
                            bias=bias_t["v"][:, oc:oc + 1], scale=1.0,
                        )
                    nc.sync.dma_start(out=v_scr[oc], in_=vs)
                    for jc in range(JC):
                        tp = tps.tile([128, 128], F32R, tag="t")
                        nc.tensor.transpose(tp, vs[:, jc * 128:(jc + 1) * 128], ident)
                        nc.vector.tensor_copy(out=vT[:, jc, oc * 128:(oc + 1) * 128], in_=tp)

        # ---------------- attention ----------------
        with tc.tile_pool(name="pp", bufs=2) as pp, \
             tc.tile_pool(name="esb", bufs=2) as esb, \
             tc.tile_pool(name="sps", bufs=3, space="PSUM") as sps, \
             tc.tile_pool(name="aps", bufs=2, space="PSUM") as aps, \
             tc.tile_pool(name="rps", bufs=1, space="PSUM") as rps, \
             tc.tile_pool(name="bps", bufs=1, space="PSUM") as bps:
            p_tiles = {}

            def emit_qk(t):
                i0, iw = IT[t]
                p_t = pp.tile([128, JC, iw], F32R, tag="p")
                p_tiles[t] = p_t
                for jc in range(JC):
                    ps = sps.tile([128, iw], F32, tag="s")
                    for ec in range(OC):
                        nc.tensor.matmul(
                            ps, k_res[:, ec, jc * 128:(jc + 1) * 128],
                            q_res[:, ec, i0:i0 + iw],
                            start=(ec == 0), stop=(ec == OC - 1),
                        )
                    nc.scalar.activation(
                        out=p_t[:, jc, :], in_=ps,
                        func=mybir.ActivationFunctionType.Exp,
                        bias=negC[:, 0:1], scale=1.0,
                    )

            def emit_post(t):
                i0, iw = IT[t]
                p_t = p_tiles.pop(t)
                rs = rps.tile([1, iw], F32, tag="rs")
                for jc in range(JC):
                    nc.tensor.matmul(rs, ones_col, p_t[:, jc, :],
                                     start=(jc == 0), stop=(jc == JC - 1))
                r_sb = esb.tile([1, iw], F32R, tag="r")
                with nc.allow_low_precision(reason="f32r recip feeds f32r matmul"):
                    nc.vector.reciprocal(out=r_sb, in_=rs)
                avs = []
                for ec in range(OC):
                    av = aps.tile([128, iw], F32, tag="av", name=f"av_{t}_{ec}")
                    for jc in range(JC):
                        nc.tensor.matmul(
                            av, vT[:, jc, ec * 128:(ec + 1) * 128], p_t[:, jc, :],
                            start=(jc == 0), stop=(jc == JC - 1),
                        )
                    avs.append(av)
                rbc = bps.tile([128, iw], F32, tag="rbc")
                nc.tensor.matmul(rbc, ones_row, r_sb, start=True, stop=True)
                rbc_sb = esb.tile([128, iw], F32, tag="rbcs")
                nc.vector.tensor_copy(out=rbc_sb, in_=rbc)
                for ec in range(OC):
                    vs_t = esb.tile([128, iw], F32R, tag="vstream", name=f"vst_{t}_{ec}")
                    nc.sync.dma_start(out=vs_t, in_=v_scr[ec, :, i0:i0 + iw])
                    o_t = esb.tile([128, iw], F32, tag="o", name=f"o_{t}_{ec}")
                    nc.vector.tensor_tensor(o_t, avs[ec], rbc_sb, mybir.AluOpType.mult)
                    nc.vector.tensor_tensor(o_t, o_t, vs_t, mybir.AluOpType.add)
                    nc.sync.dma_start(out=out_ap[ec, :, i0:i0 + iw], in_=o_t)

            emit_qk(0)
            for t in range(1, len(IT)):
                emit_qk(t)
                emit_post(t - 1)
            emit_post(len(IT) - 1)

    nc.compile()
    return nc


def _prep_shared(Wq, bq, Wk, bk, Wv, bv):
    def wprep(Wm):
        A = Wm.reshape(OC, 128, CC, 128, 3, 3)
        Bm = A.transpose(0, 3, 2, 4, 5, 1)      # [oc, c, cc, ky, kx, o]
        return np.ascontiguousarray(Bm.reshape(OC, 128, CC, 9, 128), dtype=np.float32)

    def bprep(bm):
        return np.ascontiguousarray(bm.reshape(OC, 128).T, dtype=np.float32)

    return {
        "wq": wprep(Wq), "wk": wprep(Wk), "wv": wprep(Wv),
        "bq": bprep(bq), "bk": bprep(bk), "bv": bprep(bv),
    }


def kernel(feat, Wq, bq, Wk, bk, Wv, bv):
    feat = np.asarray(feat, dtype=np.float32)
    if "nc" not in _CACHE:
        _CACHE["nc"] = _build()
    nc = _CACHE["nc"]

    shared = _prep_shared(np.asarray(Wq, np.float32), np.asarray(bq, np.float32),
                          np.asarray(Wk, np.float32), np.asarray(bk, np.float32),
                          np.asarray(Wv, np.float32), np.asarray(bv, np.float32))

    in_maps = []
    for b in range(B):
        xpad = np.zeros((C, 50, 50), np.float32)
        xpad[:, 1:49, 1:49] = feat[b]
        xpad = np.ascontiguousarray(
            xpad.reshape(CC, 128, 2500).transpose(1, 0, 2)
        )
        in_maps.append({"xpad": xpad, **shared})

    r = bass_utils.run_bass_kernel_spmd(nc, in_maps, list(range(B)))
    out = np.stack(
        [r.results[b]["out"].reshape(E, H, W) for b in range(B)], axis=0
    )
    return out



# revision 13
# speedup vs baseline: 1.2233x; 1.2233x over previous
"""ConvSA kernel for Trainium2 (8 NeuronCores, data-parallel over batch).

Computes, per batch element b (one per core):
    q/k/v = conv3x3(feat, W{q,k,v}) + b{q,k,v}        # 256 -> 512 ch, SAME pad
    att   = softmax_j(q^T k);  out = v @ att^T + v    # N = 48*48 = 2304

Convs use 1D Winograd F(2,3) along the x axis: the kx contraction of the
3x3 kernel is replaced by 4 transformed taps over stride-2 tiles, cutting
tensor-engine columns per conv from 18*N to 24*N*(24/48) = 12*N (1.5x).
Weight transform (G w) is done on host; input transform (B^T d) and the
output recombination (A^T m) are cheap DVE tensor_tensor ops.  All matmuls
in float32r.  Attention in the s^T[j, i] orientation with a single global
shift constant C (softmax is shift-invariant; C from a sampled block is
safe in fp32).  Row sums are accumulated pre-broadcast via an all-ones
[128,128] stationary matrix so the reciprocal runs as a full-width
[128,iw] DVE op instead of a slow single-partition one.
"""
import numpy as np
from contextlib import ExitStack

import concourse.bass as bass
import concourse.tile as tile
from concourse import bacc, bass_isa, bass_utils, mybir
from concourse.masks import make_identity

F32 = mybir.dt.float32
F32R = mybir.dt.float32r

B, C, H, W = 8, 256, 48, 48
E = 512
N = H * W            # 2304
CC = C // 128        # 2 c-chunks
OC = E // 128        # 4 o-chunks / e-chunks
JC = N // 128        # 18 j-chunks
TX = 24              # winograd tiles along x (F(2,3), stride 2 over 50 padded cols)
RT = [(0, 16), (16, 16), (32, 16)]   # conv row tiles -> 384-col matmuls
IT = [(0, 512), (512, 512), (1024, 512), (1536, 512), (2048, 256)]  # i tiles

_CACHE = {}


def _build():
    nc = bacc.Bacc("TRN2", target_bir_lowering=False, debug=False, num_devices=B)

    xp_ap = nc.dram_tensor("xpad", [128, CC, 2500], F32R, kind="ExternalInput").ap()
    w_aps = {
        cn: nc.dram_tensor(f"w{cn}", [OC, 128, CC, 3, 4, 128], F32R,
                           kind="ExternalInput").ap()
        for cn in "qkv"
    }
    b_aps = {
        cn: nc.dram_tensor(f"b{cn}", [128, OC], F32, kind="ExternalInput").ap()
        for cn in "qkv"
    }
    out_ap = nc.dram_tensor("out", [OC, 128, N], F32, kind="ExternalOutput").ap()

    with tile.TileContext(nc) as tc, ExitStack() as ctx:
        res = ctx.enter_context(tc.tile_pool(name="res", bufs=1))
        k_res = res.tile([128, OC, N], F32R, tag="k")
        q_res = res.tile([128, OC, N], F32R, tag="q")
        vT = res.tile([128, JC, E], F32R, tag="vT")
        bias_t = {cn: res.tile([128, OC], F32, tag=f"b{cn}", name=f"bias_{cn}")
                  for cn in "qkv"}
        ones_mat = res.tile([128, 128], F32R, tag="om")
        negC = res.tile([128, 1], F32, tag="negc")
        ident = res.tile([128, 128], F32R, tag="id")

        dram = ctx.enter_context(tc.tile_pool(name="dram", bufs=1, space="DRAM"))
        v_scr = dram.tile([OC, 128, N], F32R)

        for cn in "qkv":
            nc.sync.dma_start(out=bias_t[cn], in_=b_aps[cn])

        # ---------------- conv phase (1D winograd on x) ----------------
        with tc.tile_pool(name="vpool", bufs=1) as vpl, \
             tc.tile_pool(name="w", bufs=2) as wp, \
             tc.tile_pool(name="sc", bufs=2) as scp, \
             tc.tile_pool(name="cps", bufs=6, space="PSUM") as cps:
            V = vpl.tile([128, CC, 4, 50, TX], F32R, tag="V")

            with tc.tile_pool(name="xw", bufs=1) as xwp:
                # host supplies cols de-interleaved by parity: [.., 50, 2, 25]
                xpad_t = xwp.tile([128, CC, 50, 2, 25], F32R, tag="x")
                xflat = xpad_t.rearrange("p c h two w -> p c (h two w)")
                # chunked input DMA with the first weight tile interleaved so
                # the first conv matmul can start as early as possible
                nc.sync.dma_start(out=xflat[:, :, 0:1250], in_=xp_ap[:, :, 0:1250])
                w_first = wp.tile([128, CC, 3, 4, 128], F32R, tag="w")
                nc.sync.dma_start(out=w_first, in_=w_aps["k"][0])
                nc.sync.dma_start(out=xflat[:, :, 1250:2500], in_=xp_ap[:, :, 1250:2500])

                ident_raw = xwp.tile([128, 128], F32, tag="idr")
                make_identity(nc, ident_raw)
                nc.vector.tensor_copy(out=ident, in_=ident_raw)
                ones_raw = xwp.tile([128, 128], F32, tag="onr")
                nc.vector.memset(ones_raw, 1.0)
                nc.vector.tensor_copy(out=ones_mat, in_=ones_raw)

                # input transform: V[u] = {d0-d2, d1+d2, d2-d1, d1-d3}
                A = mybir.AluOpType
                with nc.allow_low_precision(reason="winograd input transform f32r"):
                    for (r0, rr) in [(0, 25), (25, 25)]:
                        for cc in range(CC):
                            d0 = xpad_t[:, cc, r0:r0 + rr, 0, 0:TX]
                            d1 = xpad_t[:, cc, r0:r0 + rr, 1, 0:TX]
                            d2 = xpad_t[:, cc, r0:r0 + rr, 0, 1:TX + 1]
                            d3 = xpad_t[:, cc, r0:r0 + rr, 1, 1:TX + 1]
                            vs_ = V[:, cc, :, r0:r0 + rr, :]
                            nc.vector.tensor_tensor(vs_[:, 0], d0, d2, A.subtract)
                            nc.vector.tensor_tensor(vs_[:, 1], d1, d2, A.add)
                            nc.vector.tensor_tensor(vs_[:, 2], d2, d1, A.subtract)
                            nc.vector.tensor_tensor(vs_[:, 3], d1, d3, A.subtract)

            def conv(cn, sink, w0=None):
                for oc in range(OC):
                    if oc == 0 and w0 is not None:
                        w_t = w0
                    else:
                        w_t = wp.tile([128, CC, 3, 4, 128], F32R, tag="w")
                        nc.sync.dma_start(out=w_t, in_=w_aps[cn][oc])
                    for (y0, rr) in RT:
                        Ms = []
                        for u in range(4):
                            ps = cps.tile([128, rr, TX], F32, tag="m")
                            first = True
                            for cc in range(CC):
                                for ky in range(3):
                                    nc.tensor.matmul(
                                        ps, w_t[:, cc, ky, u, :],
                                        V[:, cc, u, y0 + ky:y0 + ky + rr, :],
                                        start=first,
                                        stop=(cc == CC - 1 and ky == 2),
                                    )
                                    first = False
                            Ms.append(ps)
                        sink(cn, oc, y0, rr, Ms)

            A = mybir.AluOpType

            def recombine(dv, oc, y0, rr, Ms):
                # y_even = M0+M1+M2 ; y_odd = M1-M2-M3
                # dv is [128, 2(parity), H, TX]; pixel order is parity-major
                # (host un-permutes the final output).  tensor_tensor allows at
                # most one PSUM operand, so M1 is staged through SBUF by the
                # scalar engine.
                with nc.allow_low_precision(reason="wino recombine to f32r"):
                    a1 = scp.tile([128, rr, TX], F32, tag="a1")
                    nc.scalar.activation(
                        out=a1, in_=Ms[1],
                        func=mybir.ActivationFunctionType.Identity)
                    t0 = scp.tile([128, rr, TX], F32, tag="t0")
                    nc.vector.tensor_tensor(t0, a1, Ms[0], A.add)
                    nc.vector.tensor_tensor(
                        dv[:, 0, y0:y0 + rr, :], t0, Ms[2], A.add)
                    t1 = scp.tile([128, rr, TX], F32, tag="t1")
                    nc.vector.tensor_tensor(t1, a1, Ms[2], A.subtract)
                    nc.vector.tensor_tensor(
                        dv[:, 1, y0:y0 + rr, :], t1, Ms[3], A.subtract)

            def to_res(dst):
                dvw = dst.rearrange("p o (two y x) -> p o two y x",
                                    y=H, x=TX, two=2)

                def sink(cn, oc, y0, rr, Ms):
                    recombine(dvw[:, oc], oc, y0, rr, Ms)
                    if y0 + rr == H:  # last row tile: apply bias to whole oc slice
                        nc.scalar.activation(
                            out=dst[:, oc, :], in_=dst[:, oc, :],
                            func=mybir.ActivationFunctionType.Identity,
                            bias=bias_t[cn][:, oc:oc + 1], scale=1.0,
                        )
                return sink

            conv("k", to_res(k_res), w0=w_first)
            conv("q", to_res(q_res))

            # ---- global shift constant C (sampled block of s) ----
            # softmax is shift-invariant; any C with rowmax-80 <= C <= gmax+88
            # keeps exp() in fp32 range, so a sampled-block max works.
            with tc.tile_pool(name="mps", bufs=1, space="PSUM") as mps, \
                 tc.tile_pool(name="msb", bufs=1) as msb:
                mini = mps.tile([128, 4, 256], F32)
                for jc in range(4):
                    for ec in range(OC):
                        nc.tensor.matmul(
                            mini[:, jc, :], k_res[:, ec, jc * 128:(jc + 1) * 128],
                            q_res[:, ec, 0:256], start=(ec == 0), stop=(ec == OC - 1),
                        )
                m1 = msb.tile([128, 1], F32, tag="m1")
                nc.vector.reduce_max(out=m1, in_=mini, axis=mybir.AxisListType.XY)
                gmax = msb.tile([128, 1], F32, tag="gmax")
                nc.gpsimd.partition_all_reduce(
                    gmax, m1, channels=128, reduce_op=bass_isa.ReduceOp.max)
                nc.vector.tensor_scalar_mul(negC, gmax, -1.0)

            # v conv: stage per o-chunk, DMA to scratch + transpose into vT
            with tc.tile_pool(name="vst", bufs=2) as vstp, \
                 tc.tile_pool(name="tps", bufs=2, space="PSUM") as tps:
                def v_sink_factory():
                    state = {}

                    def sink(cn, oc, y0, rr, Ms):
                        if y0 == 0:
                            state["vs"] = vstp.tile([128, 2, H, TX], F32R,
                                                    tag="vs", name="vs")
                        vs = state["vs"]
                        recombine(vs, oc, y0, rr, Ms)
                        if y0 + rr == H:
                            vsf = vs.rearrange("p two y x -> p (two y x)")
                            nc.scalar.activation(
                                out=vsf, in_=vsf,
                                func=mybir.ActivationFunctionType.Identity,
                                bias=bias_t["v"][:, oc:oc + 1], scale=1.0,
                            )
                            nc.sync.dma_start(out=v_scr[oc], in_=vsf)
                            for jc in range(JC):
                                tp = tps.tile([128, 128], F32R, tag="t")
                                nc.tensor.transpose(
                                    tp, vsf[:, jc * 128:(jc + 1) * 128], ident)
                                nc.vector.tensor_copy(
                                    out=vT[:, jc, oc * 128:(oc + 1) * 128], in_=tp)
                    return sink

                conv("v", v_sink_factory())

        # ---------------- attention ----------------
        with tc.tile_pool(name="pp", bufs=2) as pp, \
             tc.tile_pool(name="esb", bufs=2) as esb, \
             tc.tile_pool(name="sps", bufs=3, space="PSUM") as sps, \
             tc.tile_pool(name="aps", bufs=4, space="PSUM") as aps, \
             tc.tile_pool(name="rps", bufs=1, space="PSUM") as rps:
            p_tiles = {}

            def emit_qk(t):
                i0, iw = IT[t]
                p_t = pp.tile([128, JC, iw], F32R, tag="p")
                p_tiles[t] = p_t
                for jc in range(JC):
                    ps = sps.tile([128, iw], F32, tag="s")
                    for ec in range(OC):
                        nc.tensor.matmul(
                            ps, k_res[:, ec, jc * 128:(jc + 1) * 128],
                            q_res[:, ec, i0:i0 + iw],
                            start=(ec == 0), stop=(ec == OC - 1),
                        )
                    nc.scalar.activation(
                        out=p_t[:, jc, :], in_=ps,
                        func=mybir.ActivationFunctionType.Exp,
                        bias=negC[:, 0:1], scale=1.0,
                    )

            def emit_post(t):
                i0, iw = IT[t]
                p_t = p_tiles.pop(t)
                # row sums, pre-broadcast to all 128 partitions via ones matrix
                rs = rps.tile([128, iw], F32, tag="rs")
                for jc in range(JC):
                    nc.tensor.matmul(rs, ones_mat, p_t[:, jc, :],
                                     start=(jc == 0), stop=(jc == JC - 1))
                r_inv = esb.tile([128, iw], F32R, tag="r")
                with nc.allow_low_precision(reason="f32r recip feeds f32r mult"):
                    nc.vector.reciprocal(out=r_inv, in_=rs)
                avs = []
                for ec in range(OC):
                    av = aps.tile([128, iw], F32, tag="av", name=f"av_{t}_{ec}")
                    for jc in range(JC):
                        nc.tensor.matmul(
                            av, vT[:, jc, ec * 128:(ec + 1) * 128], p_t[:, jc, :],
                            start=(jc == 0), stop=(jc == JC - 1),
                        )
                    avs.append(av)
                for ec in range(OC):
                    vs_t = esb.tile([128, iw], F32R, tag="vstream",
                                    name=f"vst_{t}_{ec}")
                    nc.sync.dma_start(out=vs_t, in_=v_scr[ec, :, i0:i0 + iw])
                    o_t = esb.tile([128, iw], F32, tag="o", name=f"o_{t}_{ec}")
                    nc.vector.tensor_tensor(o_t, avs[ec], r_inv,
                                            mybir.AluOpType.mult)
                    nc.vector.tensor_tensor(o_t, o_t, vs_t, mybir.AluOpType.add)
                    nc.sync.dma_start(out=out_ap[ec, :, i0:i0 + iw], in_=o_t)

            emit_qk(0)
            for t in range(1, len(IT)):
                emit_qk(t)
                emit_post(t - 1)
            emit_post(len(IT) - 1)

    nc.compile()
    return nc


def _prep_shared(Wq, bq, Wk, bk, Wv, bv):
    def wprep(Wm):
        Am = Wm.reshape(OC, 128, CC, 128, 3, 3).astype(np.float64)
        w0, w1, w2 = Am[..., 0], Am[..., 1], Am[..., 2]   # [oc, o, cc, c, ky]
        U = np.stack([w0, (w0 + w1 + w2) * 0.5, (w0 - w1 + w2) * 0.5, w2],
                     axis=-1)                              # [oc, o, cc, c, ky, u]
        U = U.transpose(0, 3, 2, 4, 5, 1)                  # [oc, c, cc, ky, u, o]
        return np.ascontiguousarray(
            U.reshape(OC, 128, CC, 3, 4, 128), dtype=np.float32)

    def bprep(bm):
        return np.ascontiguousarray(bm.reshape(OC, 128).T, dtype=np.float32)

    return {
        "wq": wprep(Wq), "wk": wprep(Wk), "wv": wprep(Wv),
        "bq": bprep(bq), "bk": bprep(bk), "bv": bprep(bv),
    }


def kernel(feat, Wq, bq, Wk, bk, Wv, bv):
    feat = np.asarray(feat, dtype=np.float32)
    if "nc" not in _CACHE:
        _CACHE["nc"] = _build()
    nc = _CACHE["nc"]

    shared = _prep_shared(np.asarray(Wq, np.float32), np.asarray(bq, np.float32),
                          np.asarray(Wk, np.float32), np.asarray(bk, np.float32),
                          np.asarray(Wv, np.float32), np.asarray(bv, np.float32))

    in_maps = []
    for b in range(B):
        xpad = np.zeros((C, 50, 50), np.float32)
        xpad[:, 1:49, 1:49] = feat[b]
        # de-interleave columns by parity: [C, 50, 2, 25]
        xpad = xpad.reshape(C, 50, 25, 2).transpose(0, 1, 3, 2)
        xpad = np.ascontiguousarray(
            xpad.reshape(CC, 128, 2500).transpose(1, 0, 2)
        )
        in_maps.append({"xpad": xpad, **shared})

    r = bass_utils.run_bass_kernel_spmd(nc, in_maps, list(range(B)))
    # device pixel order is parity-major [2, H, TX]; un-permute to [H, W]
    out = np.stack(
        [r.results[b]["out"].reshape(E, 2, H, TX).transpose(0, 2, 3, 1)
         .reshape(E, H, W) for b in range(B)], axis=0
    )
    return out


# revision 16
# speedup vs baseline: 1.2257x; 1.0020x over previous
"""ConvSA kernel for Trainium2 (8 NeuronCores, data-parallel over batch).

Computes, per batch element b (one per core):
    q/k/v = conv3x3(feat, W{q,k,v}) + b{q,k,v}        # 256 -> 512 ch, SAME pad
    att   = softmax_j(q^T k);  out = v @ att^T + v    # N = 48*48 = 2304

Convs use 1D Winograd F(2,3) along the x axis: the kx contraction of the
3x3 kernel is replaced by 4 transformed taps over stride-2 tiles, cutting
tensor-engine columns per conv from 18*N to 24*N*(24/48) = 12*N (1.5x).
Weight transform (G w) is done on host; input transform (B^T d) and the
output recombination (A^T m) are cheap DVE tensor_tensor ops.  All matmuls
in float32r.  Attention in the s^T[j, i] orientation with a single global
shift constant C (softmax is shift-invariant; C from a sampled block is
safe in fp32).  Row sums are accumulated pre-broadcast via an all-ones
[128,128] stationary matrix so the reciprocal runs as a full-width
[128,iw] DVE op instead of a slow single-partition one.
"""
import numpy as np
from contextlib import ExitStack

import concourse.bass as bass
import concourse.tile as tile
from concourse import bacc, bass_isa, bass_utils, mybir
from concourse.masks import make_identity

F32 = mybir.dt.float32
F32R = mybir.dt.float32r

B, C, H, W = 8, 256, 48, 48
E = 512
N = H * W            # 2304
CC = C // 128        # 2 c-chunks
OC = E // 128        # 4 o-chunks / e-chunks
JC = N // 128        # 18 j-chunks
TX = 24              # winograd tiles along x (F(2,3), stride 2 over 50 padded cols)
RT = [(0, 16), (16, 16), (32, 16)]   # conv row tiles -> 384-col matmuls
IT = [(0, 512), (512, 512), (1024, 512), (1536, 512), (2048, 256)]  # i tiles

_CACHE = {}


def _build():
    nc = bacc.Bacc("TRN2", target_bir_lowering=False, debug=False, num_devices=B)

    xp_ap = nc.dram_tensor("xpad", [128, CC, 2500], F32R, kind="ExternalInput").ap()
    w_aps = {
        cn: nc.dram_tensor(f"w{cn}", [OC, 128, CC, 3, 4, 128], F32R,
                           kind="ExternalInput").ap()
        for cn in "qkv"
    }
    b_aps = {
        cn: nc.dram_tensor(f"b{cn}", [128, OC], F32, kind="ExternalInput").ap()
        for cn in "qkv"
    }
    out_ap = nc.dram_tensor("out", [OC, 128, N], F32, kind="ExternalOutput").ap()

    with tile.TileContext(nc) as tc, ExitStack() as ctx:
        res = ctx.enter_context(tc.tile_pool(name="res", bufs=1))
        k_res = res.tile([128, OC, N], F32R, tag="k")
        q_res = res.tile([128, OC, N], F32R, tag="q")
        vT = res.tile([128, JC, E], F32R, tag="vT")
        bias_t = {cn: res.tile([128, OC], F32, tag=f"b{cn}", name=f"bias_{cn}")
                  for cn in "qkv"}
        ones_mat = res.tile([128, 128], F32R, tag="om")
        negC = res.tile([128, 1], F32, tag="negc")
        ident = res.tile([128, 128], F32R, tag="id")

        dram = ctx.enter_context(tc.tile_pool(name="dram", bufs=1, space="DRAM"))
        v_scr = dram.tile([OC, 128, N], F32R)

        for cn in "qkv":
            nc.sync.dma_start(out=bias_t[cn], in_=b_aps[cn])

        # ---------------- conv phase (1D winograd on x) ----------------
        with tc.tile_pool(name="vpool", bufs=1) as vpl, \
             tc.tile_pool(name="w", bufs=2) as wp, \
             tc.tile_pool(name="sc", bufs=2) as scp, \
             tc.tile_pool(name="cps", bufs=6, space="PSUM") as cps:
            V = vpl.tile([128, CC, 4, 50, TX], F32R, tag="V")

            with tc.tile_pool(name="xw", bufs=1) as xwp:
                # host supplies cols de-interleaved by parity: [.., 50, 2, 25]
                xpad_t = xwp.tile([128, CC, 50, 2, 25], F32R, tag="x")
                xflat = xpad_t.rearrange("p c h two w -> p c (h two w)")
                # chunked input DMA with the first weight tile interleaved so
                # the first conv matmul can start as early as possible
                nc.sync.dma_start(out=xflat[:, :, 0:1250], in_=xp_ap[:, :, 0:1250])
                w_first = wp.tile([128, CC, 3, 4, 128], F32R, tag="w")
                nc.sync.dma_start(out=w_first, in_=w_aps["k"][0])
                nc.sync.dma_start(out=xflat[:, :, 1250:2500], in_=xp_ap[:, :, 1250:2500])

                ident_raw = xwp.tile([128, 128], F32, tag="idr")
                make_identity(nc, ident_raw)
                nc.vector.tensor_copy(out=ident, in_=ident_raw)
                ones_raw = xwp.tile([128, 128], F32, tag="onr")
                nc.vector.memset(ones_raw, 1.0)
                nc.vector.tensor_copy(out=ones_mat, in_=ones_raw)

                # input transform: V[u] = {d0-d2, d1+d2, d2-d1, d1-d3}
                A = mybir.AluOpType
                with nc.allow_low_precision(reason="winograd input transform f32r"):
                    for (r0, rr) in [(0, 25), (25, 25)]:
                        for cc in range(CC):
                            d0 = xpad_t[:, cc, r0:r0 + rr, 0, 0:TX]
                            d1 = xpad_t[:, cc, r0:r0 + rr, 1, 0:TX]
                            d2 = xpad_t[:, cc, r0:r0 + rr, 0, 1:TX + 1]
                            d3 = xpad_t[:, cc, r0:r0 + rr, 1, 1:TX + 1]
                            vs_ = V[:, cc, :, r0:r0 + rr, :]
                            nc.vector.tensor_tensor(vs_[:, 0], d0, d2, A.subtract)
                            nc.vector.tensor_tensor(vs_[:, 1], d1, d2, A.add)
                            nc.vector.tensor_tensor(vs_[:, 2], d2, d1, A.subtract)
                            nc.vector.tensor_tensor(vs_[:, 3], d1, d3, A.subtract)

            def conv(cn, sink, w0=None):
                for oc in range(OC):
                    if oc == 0 and w0 is not None:
                        w_t = w0
                    else:
                        w_t = wp.tile([128, CC, 3, 4, 128], F32R, tag="w")
                        nc.sync.dma_start(out=w_t, in_=w_aps[cn][oc])
                    for (y0, rr) in RT:
                        Ms = []
                        for u in range(4):
                            ps = cps.tile([128, rr, TX], F32, tag="m")
                            first = True
                            for cc in range(CC):
                                for ky in range(3):
                                    nc.tensor.matmul(
                                        ps, w_t[:, cc, ky, u, :],
                                        V[:, cc, u, y0 + ky:y0 + ky + rr, :],
                                        start=first,
                                        stop=(cc == CC - 1 and ky == 2),
                                    )
                                    first = False
                            Ms.append(ps)
                        sink(cn, oc, y0, rr, Ms)

            A = mybir.AluOpType

            def recombine(dv, oc, y0, rr, Ms):
                # y_even = M0+M1+M2 ; y_odd = M1-M2-M3
                # dv is [128, 2(parity), H, TX]; pixel order is parity-major
                # (host un-permutes the final output).  tensor_tensor allows at
                # most one PSUM operand, so M1 is staged through SBUF by the
                # scalar engine.
                with nc.allow_low_precision(reason="wino recombine to f32r"):
                    a1 = scp.tile([128, rr, TX], F32, tag="a1")
                    nc.scalar.activation(
                        out=a1, in_=Ms[1],
                        func=mybir.ActivationFunctionType.Identity)
                    t0 = scp.tile([128, rr, TX], F32, tag="t0")
                    nc.vector.tensor_tensor(t0, a1, Ms[0], A.add)
                    nc.vector.tensor_tensor(
                        dv[:, 0, y0:y0 + rr, :], t0, Ms[2], A.add)
                    t1 = scp.tile([128, rr, TX], F32, tag="t1")
                    nc.vector.tensor_tensor(t1, a1, Ms[2], A.subtract)
                    nc.vector.tensor_tensor(
                        dv[:, 1, y0:y0 + rr, :], t1, Ms[3], A.subtract)

            def to_res(dst):
                dvw = dst.rearrange("p o (two y x) -> p o two y x",
                                    y=H, x=TX, two=2)

                def sink(cn, oc, y0, rr, Ms):
                    recombine(dvw[:, oc], oc, y0, rr, Ms)
                    if y0 + rr == H:  # last row tile: apply bias to whole oc slice
                        nc.scalar.activation(
                            out=dst[:, oc, :], in_=dst[:, oc, :],
                            func=mybir.ActivationFunctionType.Identity,
                            bias=bias_t[cn][:, oc:oc + 1], scale=1.0,
                        )
                return sink

            conv("k", to_res(k_res), w0=w_first)
            conv("q", to_res(q_res))

            # ---- global shift constant C (sampled block of s) ----
            # softmax is shift-invariant; any C with rowmax-80 <= C <= gmax+88
            # keeps exp() in fp32 range, so a sampled-block max works.
            with tc.tile_pool(name="mps", bufs=1, space="PSUM") as mps, \
                 tc.tile_pool(name="msb", bufs=1) as msb:
                mini = mps.tile([128, 4, 256], F32)
                for jc in range(4):
                    for ec in range(OC):
                        nc.tensor.matmul(
                            mini[:, jc, :], k_res[:, ec, jc * 128:(jc + 1) * 128],
                            q_res[:, ec, 0:256], start=(ec == 0), stop=(ec == OC - 1),
                        )
                m1 = msb.tile([128, 1], F32, tag="m1")
                nc.vector.reduce_max(out=m1, in_=mini, axis=mybir.AxisListType.XY)
                gmax = msb.tile([128, 1], F32, tag="gmax")
                nc.gpsimd.partition_all_reduce(
                    gmax, m1, channels=128, reduce_op=bass_isa.ReduceOp.max)
                # negate on gpsimd: keeps the slow cross-engine round-trip off
                # the in-order DVE queue (only the scalar-engine exp reads negC)
                nc.gpsimd.tensor_scalar_mul(negC, gmax, -1.0)

            # v conv: stage per o-chunk, DMA to scratch + transpose into vT
            with tc.tile_pool(name="vst", bufs=2) as vstp, \
                 tc.tile_pool(name="tps", bufs=2, space="PSUM") as tps:
                def v_sink_factory():
                    state = {}

                    def sink(cn, oc, y0, rr, Ms):
                        if y0 == 0:
                            state["vs"] = vstp.tile([128, 2, H, TX], F32R,
                                                    tag="vs", name="vs")
                        vs = state["vs"]
                        recombine(vs, oc, y0, rr, Ms)
                        if y0 + rr == H:
                            vsf = vs.rearrange("p two y x -> p (two y x)")
                            nc.scalar.activation(
                                out=vsf, in_=vsf,
                                func=mybir.ActivationFunctionType.Identity,
                                bias=bias_t["v"][:, oc:oc + 1], scale=1.0,
                            )
                            nc.sync.dma_start(out=v_scr[oc], in_=vsf)
                            for jc in range(JC):
                                tp = tps.tile([128, 128], F32R, tag="t")
                                nc.tensor.transpose(
                                    tp, vsf[:, jc * 128:(jc + 1) * 128], ident)
                                nc.scalar.activation(
                                    out=vT[:, jc, oc * 128:(oc + 1) * 128], in_=tp,
                                    func=mybir.ActivationFunctionType.Identity)
                    return sink

                conv("v", v_sink_factory())

        # ---------------- attention ----------------
        with tc.tile_pool(name="pp", bufs=2) as pp, \
             tc.tile_pool(name="esb", bufs=2) as esb, \
             tc.tile_pool(name="sps", bufs=3, space="PSUM") as sps, \
             tc.tile_pool(name="aps", bufs=4, space="PSUM") as aps, \
             tc.tile_pool(name="rps", bufs=1, space="PSUM") as rps:
            p_tiles = {}

            def emit_qk(t):
                i0, iw = IT[t]
                p_t = pp.tile([128, JC, iw], F32R, tag="p")
                p_tiles[t] = p_t
                for jc in range(JC):
                    ps = sps.tile([128, iw], F32, tag="s")
                    for ec in range(OC):
                        nc.tensor.matmul(
                            ps, k_res[:, ec, jc * 128:(jc + 1) * 128],
                            q_res[:, ec, i0:i0 + iw],
                            start=(ec == 0), stop=(ec == OC - 1),
                        )
                    nc.scalar.activation(
                        out=p_t[:, jc, :], in_=ps,
                        func=mybir.ActivationFunctionType.Exp,
                        bias=negC[:, 0:1], scale=1.0,
                    )

            def emit_post(t):
                i0, iw = IT[t]
                p_t = p_tiles.pop(t)
                # row sums, pre-broadcast to all 128 partitions via ones matrix
                rs = rps.tile([128, iw], F32, tag="rs")
                for jc in range(JC):
                    nc.tensor.matmul(rs, ones_mat, p_t[:, jc, :],
                                     start=(jc == 0), stop=(jc == JC - 1))
                r_inv = esb.tile([128, iw], F32, tag="r")
                nc.vector.reciprocal_approx_fast(out=r_inv, in_=rs)
                avs = []
                for ec in range(OC):
                    av = aps.tile([128, iw], F32, tag="av", name=f"av_{t}_{ec}")
                    for jc in range(JC):
                        nc.tensor.matmul(
                            av, vT[:, jc, ec * 128:(ec + 1) * 128], p_t[:, jc, :],
                            start=(jc == 0), stop=(jc == JC - 1),
                        )
                    avs.append(av)
                for ec in range(OC):
                    vs_t = esb.tile([128, iw], F32R, tag="vstream",
                                    name=f"vst_{t}_{ec}")
                    nc.sync.dma_start(out=vs_t, in_=v_scr[ec, :, i0:i0 + iw])
                    o_t = esb.tile([128, iw], F32, tag="o", name=f"o_{t}_{ec}")
                    nc.vector.tensor_tensor(o_t, avs[ec], r_inv,
                                            mybir.AluOpType.mult)
                    nc.vector.tensor_tensor(o_t, o_t, vs_t, mybir.AluOpType.add)
                    nc.sync.dma_start(out=out_ap[ec, :, i0:i0 + iw], in_=o_t)

            emit_qk(0)
            for t in range(1, len(IT)):
                emit_qk(t)
                emit_post(t - 1)
            emit_post(len(IT) - 1)

    nc.compile()
    return nc


def _prep_shared(Wq, bq, Wk, bk, Wv, bv):
    def wprep(Wm):
        Am = Wm.reshape(OC, 128, CC, 128, 3, 3).astype(np.float64)
        w0, w1, w2 = Am[..., 0], Am[..., 1], Am[..., 2]   # [oc, o, cc, c, ky]
        U = np.stack([w0, (w0 + w1 + w2) * 0.5, (w0 - w1 + w2) * 0.5, w2],
                     axis=-1)                              # [oc, o, cc, c, ky, u]
        U = U.transpose(0, 3, 2, 4, 5, 1)                  # [oc, c, cc, ky, u, o]
        return np.ascontiguousarray(
            U.reshape(OC, 128, CC, 3, 4, 128), dtype=np.float32)

    def bprep(bm):
        return np.ascontiguousarray(bm.reshape(OC, 128).T, dtype=np.float32)

    return {
        "wq": wprep(Wq), "wk": wprep(Wk), "wv": wprep(Wv),
        "bq": bprep(bq), "bk": bprep(bk), "bv": bprep(bv),
    }


def kernel(feat, Wq, bq, Wk, bk, Wv, bv):
    feat = np.asarray(feat, dtype=np.float32)
    if "nc" not in _CACHE:
        _CACHE["nc"] = _build()
    nc = _CACHE["nc"]

    shared = _prep_shared(np.asarray(Wq, np.float32), np.asarray(bq, np.float32),
                          np.asarray(Wk, np.float32), np.asarray(bk, np.float32),
                          np.asarray(Wv, np.float32), np.asarray(bv, np.float32))

    in_maps = []
    for b in range(B):
        xpad = np.zeros((C, 50, 50), np.float32)
        xpad[:, 1:49, 1:49] = feat[b]
        # de-interleave columns by parity: [C, 50, 2, 25]
        xpad = xpad.reshape(C, 50, 25, 2).transpose(0, 1, 3, 2)
        xpad = np.ascontiguousarray(
            xpad.reshape(CC, 128, 2500).transpose(1, 0, 2)
        )
        in_maps.append({"xpad": xpad, **shared})

    r = bass_utils.run_bass_kernel_spmd(nc, in_maps, list(range(B)))
    # device pixel order is parity-major [2, H, TX]; un-permute to [H, W]
    out = np.stack(
        [r.results[b]["out"].reshape(E, 2, H, TX).transpose(0, 2, 3, 1)
         .reshape(E, H, W) for b in range(B)], axis=0
    )
    return out


# revision 20
# speedup vs baseline: 1.2383x; 1.0103x over previous
"""ConvSA kernel for Trainium2 (8 NeuronCores, data-parallel over batch).

Computes, per batch element b (one per core):
    q/k/v = conv3x3(feat, W{q,k,v}) + b{q,k,v}        # 256 -> 512 ch, SAME pad
    att   = softmax_j(q^T k);  out = v @ att^T + v    # N = 48*48 = 2304

Convs use 1D Winograd F(2,3) along the x axis: the kx contraction of the
3x3 kernel is replaced by 4 transformed taps over stride-2 tiles, cutting
tensor-engine columns per conv from 18*N to 24*N*(24/48) = 12*N (1.5x).
Weight transform (G w) is done on host; input transform (B^T d) and the
output recombination (A^T m) are cheap DVE tensor_tensor ops.  All matmuls
in float32r.  Attention in the s^T[j, i] orientation with a single global
shift constant C (softmax is shift-invariant; C from a sampled block is
safe in fp32).  Row sums are accumulated pre-broadcast via an all-ones
[128,128] stationary matrix so the reciprocal runs as a full-width
[128,iw] DVE op instead of a slow single-partition one.
"""
import numpy as np
from contextlib import ExitStack

import concourse.bass as bass
import concourse.tile as tile
from concourse import bacc, bass_isa, bass_utils, mybir
from concourse.masks import make_identity

F32 = mybir.dt.float32
F32R = mybir.dt.float32r

B, C, H, W = 8, 256, 48, 48
E = 512
N = H * W            # 2304
CC = C // 128        # 2 c-chunks
OC = E // 128        # 4 o-chunks / e-chunks
JC = N // 128        # 18 j-chunks
TX = 24              # winograd tiles along x (F(2,3), stride 2 over 50 padded cols)
RT = [(0, 16), (16, 16), (32, 16)]   # conv row tiles -> 384-col matmuls
IT = [(0, 512), (512, 512), (1024, 512), (1536, 512), (2048, 256)]  # i tiles

_CACHE = {}


def _build():
    nc = bacc.Bacc("TRN2", target_bir_lowering=False, debug=False, num_devices=B)

    xp_ap = nc.dram_tensor("xpad", [128, CC, 2500], F32R, kind="ExternalInput").ap()
    w_aps = {
        cn: nc.dram_tensor(f"w{cn}", [OC, 128, CC, 3, 4, 128], F32R,
                           kind="ExternalInput").ap()
        for cn in "qkv"
    }
    b_aps = {
        cn: nc.dram_tensor(f"b{cn}", [128, OC], F32, kind="ExternalInput").ap()
        for cn in "qkv"
    }
    out_ap = nc.dram_tensor("out", [OC, 128, N], F32, kind="ExternalOutput").ap()

    with tile.TileContext(nc) as tc, ExitStack() as ctx:
        res = ctx.enter_context(tc.tile_pool(name="res", bufs=1))
        k_res = res.tile([128, OC, N], F32R, tag="k")
        q_res = res.tile([128, OC, N], F32R, tag="q")
        vT = res.tile([128, JC, E], F32R, tag="vT")
        bias_t = {cn: res.tile([128, OC], F32, tag=f"b{cn}", name=f"bias_{cn}")
                  for cn in "qkv"}
        ones_mat = res.tile([128, 128], F32R, tag="om")
        negC = res.tile([128, 1], F32, tag="negc")
        ident = res.tile([128, 128], F32R, tag="id")
        # minimax scratch lives in the persistent pool: putting it in a
        # short-lived pool makes the next pool's tiles WAR-depend on the slow
        # gpsimd readers, stalling the DVE queue (and then the tensor engine)
        m1 = res.tile([128, 1], F32, tag="m1")
        gmax = res.tile([128, 1], F32, tag="gmax")

        dram = ctx.enter_context(tc.tile_pool(name="dram", bufs=1, space="DRAM"))
        v_scr = dram.tile([OC, 128, N], F32R)

        for cn in "qkv":
            nc.sync.dma_start(out=bias_t[cn], in_=b_aps[cn])

        # ---------------- conv phase (1D winograd on x) ----------------
        with tc.tile_pool(name="vpool", bufs=1) as vpl, \
             tc.tile_pool(name="w", bufs=2) as wp, \
             tc.tile_pool(name="sc", bufs=2) as scp, \
             tc.tile_pool(name="cps", bufs=6, space="PSUM") as cps:
            V = vpl.tile([128, CC, 4, 50, TX], F32R, tag="V")

            with tc.tile_pool(name="xw", bufs=1) as xwp:
                # host supplies cols de-interleaved by parity: [.., 50, 2, 25]
                xpad_t = xwp.tile([128, CC, 50, 2, 25], F32R, tag="x")
                xflat = xpad_t.rearrange("p c h two w -> p c (h two w)")
                # chunked input DMA with the first weight tile interleaved so
                # the first conv matmul can start as early as possible
                nc.sync.dma_start(out=xflat[:, :, 0:950], in_=xp_ap[:, :, 0:950])
                w_first = wp.tile([128, CC, 3, 4, 128], F32R, tag="w")
                nc.sync.dma_start(out=w_first, in_=w_aps["k"][0])
                nc.sync.dma_start(out=xflat[:, :, 950:2500], in_=xp_ap[:, :, 950:2500])

                # input transform: V[u] = {d0-d2, d1+d2, d2-d1, d1-d3}
                A = mybir.AluOpType
                with nc.allow_low_precision(reason="winograd input transform f32r"):
                    for (r0, rr) in [(0, 19), (19, 31)]:
                        for cc in range(CC):
                            d0 = xpad_t[:, cc, r0:r0 + rr, 0, 0:TX]
                            d1 = xpad_t[:, cc, r0:r0 + rr, 1, 0:TX]
                            d2 = xpad_t[:, cc, r0:r0 + rr, 0, 1:TX + 1]
                            d3 = xpad_t[:, cc, r0:r0 + rr, 1, 1:TX + 1]
                            vs_ = V[:, cc, :, r0:r0 + rr, :]
                            nc.vector.tensor_tensor(vs_[:, 0], d0, d2, A.subtract)
                            nc.vector.tensor_tensor(vs_[:, 1], d1, d2, A.add)
                            nc.vector.tensor_tensor(vs_[:, 2], d2, d1, A.subtract)
                            nc.vector.tensor_tensor(vs_[:, 3], d1, d3, A.subtract)

                # engine-setup constants, after the transforms so they don't
                # delay the first conv matmul on the in-order DVE queue
                ident_raw = xwp.tile([128, 128], F32, tag="idr")
                make_identity(nc, ident_raw)
                nc.vector.tensor_copy(out=ident, in_=ident_raw)
                ones_raw = xwp.tile([128, 128], F32, tag="onr")
                nc.vector.memset(ones_raw, 1.0)
                nc.vector.tensor_copy(out=ones_mat, in_=ones_raw)

            def conv(cn, sink, w0=None):
                for oc in range(OC):
                    if oc == 0 and w0 is not None:
                        w_t = w0
                    else:
                        w_t = wp.tile([128, CC, 3, 4, 128], F32R, tag="w")
                        nc.sync.dma_start(out=w_t, in_=w_aps[cn][oc])
                    for (y0, rr) in RT:
                        Ms = []
                        for u in range(4):
                            ps = cps.tile([128, rr, TX], F32, tag="m")
                            first = True
                            for cc in range(CC):
                                for ky in range(3):
                                    nc.tensor.matmul(
                                        ps, w_t[:, cc, ky, u, :],
                                        V[:, cc, u, y0 + ky:y0 + ky + rr, :],
                                        start=first,
                                        stop=(cc == CC - 1 and ky == 2),
                                    )
                                    first = False
                            Ms.append(ps)
                        sink(cn, oc, y0, rr, Ms)

            A = mybir.AluOpType

            def recombine(dv, oc, y0, rr, Ms):
                # y_even = M0+M1+M2 ; y_odd = M1-M2-M3
                # dv is [128, 2(parity), H, TX]; pixel order is parity-major
                # (host un-permutes the final output).  tensor_tensor allows at
                # most one PSUM operand, so M1 is staged through SBUF by the
                # scalar engine.
                with nc.allow_low_precision(reason="wino recombine to f32r"):
                    a1 = scp.tile([128, rr, TX], F32, tag="a1")
                    nc.scalar.activation(
                        out=a1, in_=Ms[1],
                        func=mybir.ActivationFunctionType.Identity)
                    t0 = scp.tile([128, rr, TX], F32, tag="t0")
                    nc.vector.tensor_tensor(t0, a1, Ms[0], A.add)
                    nc.vector.tensor_tensor(
                        dv[:, 0, y0:y0 + rr, :], t0, Ms[2], A.add)
                    t1 = scp.tile([128, rr, TX], F32, tag="t1")
                    nc.vector.tensor_tensor(t1, a1, Ms[2], A.subtract)
                    nc.vector.tensor_tensor(
                        dv[:, 1, y0:y0 + rr, :], t1, Ms[3], A.subtract)

            def to_res(dst):
                dvw = dst.rearrange("p o (two y x) -> p o two y x",
                                    y=H, x=TX, two=2)

                def sink(cn, oc, y0, rr, Ms):
                    recombine(dvw[:, oc], oc, y0, rr, Ms)
                    if y0 + rr == H:  # last row tile: apply bias to whole oc slice
                        nc.scalar.activation(
                            out=dst[:, oc, :], in_=dst[:, oc, :],
                            func=mybir.ActivationFunctionType.Identity,
                            bias=bias_t[cn][:, oc:oc + 1], scale=1.0,
                        )
                return sink

            conv("k", to_res(k_res), w0=w_first)
            conv("q", to_res(q_res))

            # ---- global shift constant C (sampled block of s) ----
            # softmax is shift-invariant; any C with rowmax-80 <= C <= gmax+88
            # keeps exp() in fp32 range, so a sampled-block max works.
            with tc.tile_pool(name="mps", bufs=1, space="PSUM") as mps:
                mini = mps.tile([128, 2, 256], F32)
                for jc in range(2):
                    for ec in range(OC):
                        nc.tensor.matmul(
                            mini[:, jc, :], k_res[:, ec, jc * 128:(jc + 1) * 128],
                            q_res[:, ec, 0:256], start=(ec == 0), stop=(ec == OC - 1),
                        )
                nc.vector.reduce_max(out=m1, in_=mini, axis=mybir.AxisListType.XY)
                nc.gpsimd.partition_all_reduce(
                    gmax, m1, channels=128, reduce_op=bass_isa.ReduceOp.max)
                # negate on gpsimd: keeps the slow cross-engine round-trip off
                # the in-order DVE queue (only the scalar-engine exp reads negC)
                nc.gpsimd.tensor_scalar_mul(negC, gmax, -1.0)

            # v conv: stage per o-chunk, DMA to scratch + transpose into vT
            with tc.tile_pool(name="vst", bufs=2) as vstp, \
                 tc.tile_pool(name="tps", bufs=2, space="PSUM") as tps:
                def v_sink_factory():
                    state = {}

                    def sink(cn, oc, y0, rr, Ms):
                        if y0 == 0:
                            state["vs"] = vstp.tile([128, 2, H, TX], F32R,
                                                    tag="vs", name="vs")
                        vs = state["vs"]
                        recombine(vs, oc, y0, rr, Ms)
                        if y0 + rr == H:
                            vsf = vs.rearrange("p two y x -> p (two y x)")
                            nc.scalar.activation(
                                out=vsf, in_=vsf,
                                func=mybir.ActivationFunctionType.Identity,
                                bias=bias_t["v"][:, oc:oc + 1], scale=1.0,
                            )
                            nc.sync.dma_start(out=v_scr[oc], in_=vsf)
                            for jc in range(JC):
                                tp = tps.tile([128, 128], F32R, tag="t")
                                nc.tensor.transpose(
                                    tp, vsf[:, jc * 128:(jc + 1) * 128], ident)
                                nc.scalar.activation(
                                    out=vT[:, jc, oc * 128:(oc + 1) * 128], in_=tp,
                                    func=mybir.ActivationFunctionType.Identity)
                    return sink

                conv("v", v_sink_factory())

        # ---------------- attention ----------------
        with tc.tile_pool(name="pp", bufs=2) as pp, \
             tc.tile_pool(name="esb", bufs=2) as esb, \
             tc.tile_pool(name="sps", bufs=3, space="PSUM") as sps, \
             tc.tile_pool(name="aps", bufs=4, space="PSUM") as aps, \
             tc.tile_pool(name="rps", bufs=1, space="PSUM") as rps:
            p_tiles = {}

            def emit_qk(t):
                i0, iw = IT[t]
                p_t = pp.tile([128, JC, iw], F32R, tag="p")
                p_tiles[t] = p_t
                for jc in range(JC):
                    ps = sps.tile([128, iw], F32, tag="s")
                    for ec in range(OC):
                        nc.tensor.matmul(
                            ps, k_res[:, ec, jc * 128:(jc + 1) * 128],
                            q_res[:, ec, i0:i0 + iw],
                            start=(ec == 0), stop=(ec == OC - 1),
                        )
                    nc.scalar.activation(
                        out=p_t[:, jc, :], in_=ps,
                        func=mybir.ActivationFunctionType.Exp,
                        bias=negC[:, 0:1], scale=1.0,
                    )

            def emit_post(t):
                i0, iw = IT[t]
                p_t = p_tiles.pop(t)
                # row sums, pre-broadcast to all 128 partitions via ones matrix
                rs = rps.tile([128, iw], F32, tag="rs")
                for jc in range(JC):
                    nc.tensor.matmul(rs, ones_mat, p_t[:, jc, :],
                                     start=(jc == 0), stop=(jc == JC - 1))
                r_inv = esb.tile([128, iw], F32, tag="r")
                nc.vector.reciprocal_approx_fast(out=r_inv, in_=rs)
                avs = []
                for ec in range(OC):
                    av = aps.tile([128, iw], F32, tag="av", name=f"av_{t}_{ec}")
                    for jc in range(JC):
                        nc.tensor.matmul(
                            av, vT[:, jc, ec * 128:(ec + 1) * 128], p_t[:, jc, :],
                            start=(jc == 0), stop=(jc == JC - 1),
                        )
                    avs.append(av)
                for ec in range(OC):
                    vs_t = esb.tile([128, iw], F32R, tag="vstream",
                                    name=f"vst_{t}_{ec}")
                    nc.sync.dma_start(out=vs_t, in_=v_scr[ec, :, i0:i0 + iw])
                    o_t = esb.tile([128, iw], F32, tag="o", name=f"o_{t}_{ec}")
                    nc.vector.tensor_tensor(o_t, avs[ec], r_inv,
                                            mybir.AluOpType.mult)
                    nc.vector.tensor_tensor(o_t, o_t, vs_t, mybir.AluOpType.add)
                    nc.sync.dma_start(out=out_ap[ec, :, i0:i0 + iw], in_=o_t)

            emit_qk(0)
            for t in range(1, len(IT)):
                emit_qk(t)
                emit_post(t - 1)
            emit_post(len(IT) - 1)

    nc.compile()
    return nc


def _prep_shared(Wq, bq, Wk, bk, Wv, bv):
    def wprep(Wm):
        Am = Wm.reshape(OC, 128, CC, 128, 3, 3).astype(np.float64)
        w0, w1, w2 = Am[..., 0], Am[..., 1], Am[..., 2]   # [oc, o, cc, c, ky]
        U = np.stack([w0, (w0 + w1 + w2) * 0.5, (w0 - w1 + w2) * 0.5, w2],
                     axis=-1)                              # [oc, o, cc, c, ky, u]
        U = U.transpose(0, 3, 2, 4, 5, 1)                  # [oc, c, cc, ky, u, o]
        return np.ascontiguousarray(
            U.reshape(OC, 128, CC, 3, 4, 128), dtype=np.float32)

    def bprep(bm):
        return np.ascontiguousarray(bm.reshape(OC, 128).T, dtype=np.float32)

    return {
        "wq": wprep(Wq), "wk": wprep(Wk), "wv": wprep(Wv),
        "bq": bprep(bq), "bk": bprep(bk), "bv": bprep(bv),
    }


def kernel(feat, Wq, bq, Wk, bk, Wv, bv):
    feat = np.asarray(feat, dtype=np.float32)
    if "nc" not in _CACHE:
        _CACHE["nc"] = _build()
    nc = _CACHE["nc"]

    shared = _prep_shared(np.asarray(Wq, np.float32), np.asarray(bq, np.float32),
                          np.asarray(Wk, np.float32), np.asarray(bk, np.float32),
                          np.asarray(Wv, np.float32), np.asarray(bv, np.float32))

    in_maps = []
    for b in range(B):
        xpad = np.zeros((C, 50, 50), np.float32)
        xpad[:, 1:49, 1:49] = feat[b]
        # de-interleave columns by parity: [C, 50, 2, 25]
        xpad = xpad.reshape(C, 50, 25, 2).transpose(0, 1, 3, 2)
        xpad = np.ascontiguousarray(
            xpad.reshape(CC, 128, 2500).transpose(1, 0, 2)
        )
        in_maps.append({"xpad": xpad, **shared})

    r = bass_utils.run_bass_kernel_spmd(nc, in_maps, list(range(B)))
    # device pixel order is parity-major [2, H, TX]; un-permute to [H, W]
    out = np.stack(
        [r.results[b]["out"].reshape(E, 2, H, TX).transpose(0, 2, 3, 1)
         .reshape(E, H, W) for b in range(B)], axis=0
    )
    return out


# revision 22
# speedup vs baseline: 1.3011x; 1.0507x over previous
"""ConvSA kernel for Trainium2 (8 NeuronCores, data-parallel over batch).

Computes, per batch element b (one per core):
    q/k/v = conv3x3(feat, W{q,k,v}) + b{q,k,v}        # 256 -> 512 ch, SAME pad
    att   = softmax_j(q^T k);  out = v @ att^T + v    # N = 48*48 = 2304

Convs use 1D Winograd F(2,3) along the x axis: the kx contraction of the
3x3 kernel becomes 4 transformed taps over stride-2 tiles, cutting
tensor-engine columns per conv by 1.5x.  Both transforms (G w and B^T d)
are computed on host; only the 2-op-per-parity output recombination
(A^T m) runs on device (DVE).  q/k and the attention scores stay in
float32r (the softmax here is nearly argmax - score noise flips rows);
the v path and the exp'd probabilities are bf16, which lets v stay
resident in SBUF (no DRAM round-trip) and halves p storage.  Attention
uses the s^T[j, i] orientation with one global shift constant C (softmax
is shift-invariant; a sampled-block max is safe in fp32).  Row sums are
accumulated pre-broadcast via an all-ones [128,128] stationary matrix so
the reciprocal runs as a full-width [128,iw] DVE op.
"""
import numpy as np
from contextlib import ExitStack

import concourse.bass as bass
import concourse.tile as tile
from concourse import bacc, bass_isa, bass_utils, mybir
from concourse.masks import make_identity

F32 = mybir.dt.float32
F32R = mybir.dt.float32r
BF16 = mybir.dt.bfloat16

B, C, H, W = 8, 256, 48, 48
E = 512
N = H * W            # 2304
CC = C // 128        # 2 c-chunks
OC = E // 128        # 4 o-chunks / e-chunks
JC = N // 128        # 18 j-chunks
TX = 24              # winograd tiles along x (F(2,3), stride 2 over 50 padded cols)
RT = [(0, 16), (16, 16), (32, 16)]   # conv row tiles -> 384-col matmuls
IT = [(0, 512), (512, 512), (1024, 512), (1536, 512), (2048, 256)]  # i tiles

_CACHE = {}


def _build():
    nc = bacc.Bacc("TRN2", target_bir_lowering=False, debug=False, num_devices=B)

    v_ap = nc.dram_tensor("vwino", [128, CC, 4, 50, TX], F32R,
                          kind="ExternalInput").ap()
    w_aps = {
        cn: nc.dram_tensor(f"w{cn}", [OC, 128, CC, 3, 4, 128], F32R,
                           kind="ExternalInput").ap()
        for cn in "qkv"
    }
    b_aps = {
        cn: nc.dram_tensor(f"b{cn}", [128, OC], F32, kind="ExternalInput").ap()
        for cn in "qkv"
    }
    out_ap = nc.dram_tensor("out", [OC, 128, N], F32, kind="ExternalOutput").ap()

    with tile.TileContext(nc) as tc, ExitStack() as ctx:
        res = ctx.enter_context(tc.tile_pool(name="res", bufs=1))
        k_res = res.tile([128, OC, N], F32R, tag="k")
        q_res = res.tile([128, OC, N], F32R, tag="q")
        vT = res.tile([128, JC, E], BF16, tag="vT")
        vs_all = res.tile([128, OC, 2, H, TX], BF16, tag="vs")
        bias_t = {cn: res.tile([128, OC], F32, tag=f"b{cn}", name=f"bias_{cn}")
                  for cn in "qkv"}
        ones_b = res.tile([128, 128], BF16, tag="om")
        negC = res.tile([128, 1], F32, tag="negc")
        ident_b = res.tile([128, 128], BF16, tag="id")
        # minimax scratch lives in the persistent pool: a short-lived pool
        # makes the next pool's tiles WAR-depend on the slow gpsimd readers,
        # stalling the DVE queue (and then the tensor engine)
        m1 = res.tile([128, 1], F32, tag="m1")
        gmax = res.tile([128, 1], F32, tag="gmax")

        # ---------------- conv phase (1D winograd on x) ----------------
        with tc.tile_pool(name="vpool", bufs=1) as vpl, \
             tc.tile_pool(name="w", bufs=2) as wp, \
             tc.tile_pool(name="sc", bufs=2) as scp, \
             tc.tile_pool(name="cps", bufs=6, space="PSUM") as cps:
            V = vpl.tile([128, CC, 4, 50, TX], F32R, tag="V")

            # DMA order tuned for earliest first matmul: the u=0 rows the
            # first row tile needs, then the first weight tile, then the rest
            nc.sync.dma_start(out=V[:, :, 0, 0:19, :], in_=v_ap[:, :, 0, 0:19, :])
            w_first = wp.tile([128, CC, 3, 4, 128], F32R, tag="w")
            nc.sync.dma_start(out=w_first, in_=w_aps["k"][0])
            for u in range(1, 4):
                nc.sync.dma_start(out=V[:, :, u, 0:19, :],
                                  in_=v_ap[:, :, u, 0:19, :])
            for u in range(4):
                nc.sync.dma_start(out=V[:, :, u, 19:50, :],
                                  in_=v_ap[:, :, u, 19:50, :])
            for cn in "qkv":
                nc.sync.dma_start(out=bias_t[cn], in_=b_aps[cn])

            with tc.tile_pool(name="xw", bufs=1) as xwp:
                ident_raw = xwp.tile([128, 128], F32, tag="idr")
                make_identity(nc, ident_raw)
                nc.vector.tensor_copy(out=ident_b, in_=ident_raw)
                ones_raw = xwp.tile([128, 128], F32, tag="onr")
                nc.vector.memset(ones_raw, 1.0)
                nc.vector.tensor_copy(out=ones_b, in_=ones_raw)

            def conv(cn, sink, w0=None):
                for oc in range(OC):
                    if oc == 0 and w0 is not None:
                        w_t = w0
                    else:
                        w_t = wp.tile([128, CC, 3, 4, 128], F32R, tag="w")
                        nc.sync.dma_start(out=w_t, in_=w_aps[cn][oc])
                    for (y0, rr) in RT:
                        Ms = []
                        for u in range(4):
                            ps = cps.tile([128, rr, TX], F32, tag="m")
                            first = True
                            for cc in range(CC):
                                for ky in range(3):
                                    nc.tensor.matmul(
                                        ps, w_t[:, cc, ky, u, :],
                                        V[:, cc, u, y0 + ky:y0 + ky + rr, :],
                                        start=first,
                                        stop=(cc == CC - 1 and ky == 2),
                                    )
                                    first = False
                            Ms.append(ps)
                        sink(cn, oc, y0, rr, Ms)

            A = mybir.AluOpType

            def recombine(dv, y0, rr, Ms):
                # y_even = M0+M1+M2 ; y_odd = M1-M2-M3
                # dv is [128, 2(parity), H, TX]; pixel order is parity-major
                # (host un-permutes the final output).  tensor_tensor allows at
                # most one PSUM operand, so M1 is staged through SBUF by the
                # scalar engine.
                with nc.allow_low_precision(reason="wino recombine"):
                    a1 = scp.tile([128, rr, TX], F32, tag="a1")
                    nc.scalar.activation(
                        out=a1, in_=Ms[1],
                        func=mybir.ActivationFunctionType.Identity)
                    t0 = scp.tile([128, rr, TX], F32, tag="t0")
                    nc.vector.tensor_tensor(t0, a1, Ms[0], A.add)
                    nc.vector.tensor_tensor(
                        dv[:, 0, y0:y0 + rr, :], t0, Ms[2], A.add)
                    t1 = scp.tile([128, rr, TX], F32, tag="t1")
                    nc.vector.tensor_tensor(t1, a1, Ms[2], A.subtract)
                    nc.vector.tensor_tensor(
                        dv[:, 1, y0:y0 + rr, :], t1, Ms[3], A.subtract)

            def to_res(dst):
                dvw = dst.rearrange("p o (two y x) -> p o two y x",
                                    y=H, x=TX, two=2)

                def sink(cn, oc, y0, rr, Ms):
                    recombine(dvw[:, oc], y0, rr, Ms)
                    if y0 + rr == H:  # last row tile: apply bias to whole oc slice
                        nc.scalar.activation(
                            out=dst[:, oc, :], in_=dst[:, oc, :],
                            func=mybir.ActivationFunctionType.Identity,
                            bias=bias_t[cn][:, oc:oc + 1], scale=1.0,
                        )
                return sink

            conv("k", to_res(k_res), w0=w_first)
            conv("q", to_res(q_res))

            # ---- global shift constant C (sampled block of s) ----
            # softmax is shift-invariant; any C with rowmax-80 <= C <= gmax+88
            # keeps exp() in fp32 range, so a sampled-block max works.
            with tc.tile_pool(name="mps", bufs=1, space="PSUM") as mps:
                mini = mps.tile([128, 2, 256], F32)
                for jc in range(2):
                    for ec in range(OC):
                        nc.tensor.matmul(
                            mini[:, jc, :], k_res[:, ec, jc * 128:(jc + 1) * 128],
                            q_res[:, ec, 0:256], start=(ec == 0), stop=(ec == OC - 1),
                        )
                nc.vector.reduce_max(out=m1, in_=mini, axis=mybir.AxisListType.XY)
                nc.gpsimd.partition_all_reduce(
                    gmax, m1, channels=128, reduce_op=bass_isa.ReduceOp.max)
                # negate on gpsimd: keeps the slow cross-engine round-trip off
                # the in-order DVE queue (only the scalar-engine exp reads negC)
                nc.gpsimd.tensor_scalar_mul(negC, gmax, -1.0)

            # v conv: recombine into resident bf16 vs, transpose into vT
            with tc.tile_pool(name="tps", bufs=2, space="PSUM") as tps:
                def v_sink(cn, oc, y0, rr, Ms):
                    recombine(vs_all[:, oc], y0, rr, Ms)
                    if y0 + rr == H:
                        vsf = vs_all.rearrange("p o two y x -> p o (two y x)")[:, oc]
                        nc.scalar.activation(
                            out=vsf, in_=vsf,
                            func=mybir.ActivationFunctionType.Identity,
                            bias=bias_t["v"][:, oc:oc + 1], scale=1.0,
                        )
                        for jc in range(JC):
                            tp = tps.tile([128, 128], BF16, tag="t")
                            nc.tensor.transpose(
                                tp, vsf[:, jc * 128:(jc + 1) * 128], ident_b)
                            nc.scalar.activation(
                                out=vT[:, jc, oc * 128:(oc + 1) * 128], in_=tp,
                                func=mybir.ActivationFunctionType.Identity)

                conv("v", v_sink)

        # ---------------- attention ----------------
        vs_flat = vs_all.rearrange("p o two y x -> p o (two y x)")
        with tc.tile_pool(name="pp", bufs=2) as pp, \
             tc.tile_pool(name="esb", bufs=2) as esb, \
             tc.tile_pool(name="sps", bufs=3, space="PSUM") as sps, \
             tc.tile_pool(name="aps", bufs=4, space="PSUM") as aps, \
             tc.tile_pool(name="rps", bufs=1, space="PSUM") as rps:
            p_tiles = {}

            def emit_qk(t):
                i0, iw = IT[t]
                p_t = pp.tile([128, JC, iw], BF16, tag="p")
                p_tiles[t] = p_t
                for jc in range(JC):
                    ps = sps.tile([128, iw], F32, tag="s")
                    for ec in range(OC):
                        nc.tensor.matmul(
                            ps, k_res[:, ec, jc * 128:(jc + 1) * 128],
                            q_res[:, ec, i0:i0 + iw],
                            start=(ec == 0), stop=(ec == OC - 1),
                        )
                    nc.scalar.activation(
                        out=p_t[:, jc, :], in_=ps,
                        func=mybir.ActivationFunctionType.Exp,
                        bias=negC[:, 0:1], scale=1.0,
                    )

            def emit_post(t):
                i0, iw = IT[t]
                p_t = p_tiles.pop(t)
                # row sums, pre-broadcast to all 128 partitions via ones matrix
                rs = rps.tile([128, iw], F32, tag="rs")
                for jc in range(JC):
                    nc.tensor.matmul(rs, ones_b, p_t[:, jc, :],
                                     start=(jc == 0), stop=(jc == JC - 1))
                r_inv = esb.tile([128, iw], F32, tag="r")
                nc.vector.reciprocal_approx_fast(out=r_inv, in_=rs)
                avs = []
                for ec in range(OC):
                    av = aps.tile([128, iw], F32, tag="av", name=f"av_{t}_{ec}")
                    for jc in range(JC):
                        nc.tensor.matmul(
                            av, vT[:, jc, ec * 128:(ec + 1) * 128], p_t[:, jc, :],
                            start=(jc == 0), stop=(jc == JC - 1),
                        )
                    avs.append(av)
                for ec in range(OC):
                    o_t = esb.tile([128, iw], F32, tag="o", name=f"o_{t}_{ec}")
                    nc.vector.tensor_tensor(o_t, avs[ec], r_inv,
                                            mybir.AluOpType.mult)
                    nc.vector.tensor_tensor(o_t, o_t, vs_flat[:, ec, i0:i0 + iw],
                                            mybir.AluOpType.add)
                    nc.sync.dma_start(out=out_ap[ec, :, i0:i0 + iw], in_=o_t)

            emit_qk(0)
            for t in range(1, len(IT)):
                emit_qk(t)
                emit_post(t - 1)
            emit_post(len(IT) - 1)

    nc.compile()
    return nc


def _prep_shared(Wq, bq, Wk, bk, Wv, bv):
    def wprep(Wm):
        Am = Wm.reshape(OC, 128, CC, 128, 3, 3).astype(np.float64)
        w0, w1, w2 = Am[..., 0], Am[..., 1], Am[..., 2]   # [oc, o, cc, c, ky]
        U = np.stack([w0, (w0 + w1 + w2) * 0.5, (w0 - w1 + w2) * 0.5, w2],
                     axis=-1)                              # [oc, o, cc, c, ky, u]
        U = U.transpose(0, 3, 2, 4, 5, 1)                  # [oc, c, cc, ky, u, o]
        return np.ascontiguousarray(
            U.reshape(OC, 128, CC, 3, 4, 128), dtype=np.float32)

    def bprep(bm):
        return np.ascontiguousarray(bm.reshape(OC, 128).T, dtype=np.float32)

    return {
        "wq": wprep(Wq), "wk": wprep(Wk), "wv": wprep(Wv),
        "bq": bprep(bq), "bk": bprep(bk), "bv": bprep(bv),
    }


def kernel(feat, Wq, bq, Wk, bk, Wv, bv):
    feat = np.asarray(feat, dtype=np.float32)
    if "nc" not in _CACHE:
        _CACHE["nc"] = _build()
    nc = _CACHE["nc"]

    shared = _prep_shared(np.asarray(Wq, np.float32), np.asarray(bq, np.float32),
                          np.asarray(Wk, np.float32), np.asarray(bk, np.float32),
                          np.asarray(Wv, np.float32), np.asarray(bv, np.float32))

    in_maps = []
    for b in range(B):
        xpad = np.zeros((C, 50, 50), np.float32)
        xpad[:, 1:49, 1:49] = feat[b]
        # host-side winograd input transform B^T d over stride-2 x tiles
        d0 = xpad[:, :, 0:48:2]
        d1 = xpad[:, :, 1:48:2]
        d2 = xpad[:, :, 2:50:2]
        d3 = xpad[:, :, 3:50:2]
        Vh = np.stack([d0 - d2, d1 + d2, d2 - d1, d1 - d3], axis=1)  # [C,4,50,24]
        Vh = np.ascontiguousarray(
            Vh.reshape(CC, 128, 4, 50, TX).transpose(1, 0, 2, 3, 4))
        in_maps.append({"vwino": Vh, **shared})

    r = bass_utils.run_bass_kernel_spmd(nc, in_maps, list(range(B)))
    # device pixel order is parity-major [2, H, TX]; un-permute to [H, W]
    out = np.stack(
        [r.results[b]["out"].reshape(E, 2, H, TX).transpose(0, 2, 3, 1)
         .reshape(E, H, W) for b in range(B)], axis=0
    )
    return out


# revision 28
# speedup vs baseline: 1.3753x; 1.0571x over previous
"""ConvSA kernel for Trainium2 (8 NeuronCores, data-parallel over batch).

Computes, per batch element b (one per core):
    q/k/v = conv3x3(feat, W{q,k,v}) + b{q,k,v}        # 256 -> 512 ch, SAME pad
    att   = softmax_j(q^T k);  out = v @ att^T + v    # N = 48*48 = 2304

Convs use 1D Winograd F(2,3) along the x axis: the kx contraction of the
3x3 kernel becomes 4 transformed taps over stride-2 tiles, cutting
tensor-engine columns per conv by 1.5x.  Both transforms (G w and B^T d)
are computed on host; only the 2-op-per-parity output recombination
(A^T m) runs on device (DVE).  q/k and the attention scores stay in
float32r (the softmax here is nearly argmax - score noise flips rows);
the v path and the exp'd probabilities are bf16, which lets v stay
resident in SBUF (no DRAM round-trip) and halves p storage.  Attention
uses the s^T[j, i] orientation with one global shift constant C (softmax
is shift-invariant; a sampled-block max is safe in fp32).  Row sums are
accumulated pre-broadcast via an all-ones [128,128] stationary matrix so
the reciprocal runs as a full-width [128,iw] DVE op.
"""
import ml_dtypes
import numpy as np
from contextlib import ExitStack

import concourse.bass as bass
import concourse.tile as tile
from concourse import bacc, bass_isa, bass_utils, mybir
from concourse.masks import make_identity

F32 = mybir.dt.float32
F32R = mybir.dt.float32r
BF16 = mybir.dt.bfloat16

B, C, H, W = 8, 256, 48, 48
E = 512
N = H * W            # 2304
CC = C // 128        # 2 c-chunks
OC = E // 128        # 4 o-chunks / e-chunks
JC = N // 128        # 18 j-chunks
TX = 24              # winograd tiles along x (F(2,3), stride 2 over 50 padded cols)
RT = [(0, 16), (16, 16), (32, 16)]   # conv row tiles -> 384-col matmuls
IT = [(0, 512), (512, 512), (1024, 512), (1536, 512), (2048, 256)]  # i tiles

_CACHE = {}


def _build():
    nc = bacc.Bacc("TRN2", target_bir_lowering=False, debug=False, num_devices=B)

    v_ap = nc.dram_tensor("vwino", [128, CC, 4, 50, TX], BF16,
                          kind="ExternalInput").ap()
    w_aps = {
        cn: nc.dram_tensor(f"w{cn}", [OC, 128, CC, 3, 4, 128], BF16,
                           kind="ExternalInput").ap()
        for cn in "qkv"
    }
    b_aps = {
        cn: nc.dram_tensor(f"b{cn}", [128, OC], F32, kind="ExternalInput").ap()
        for cn in "qkv"
    }
    out_ap = nc.dram_tensor("out", [OC, 128, N], F32, kind="ExternalOutput").ap()

    with tile.TileContext(nc) as tc, ExitStack() as ctx:
        res = ctx.enter_context(tc.tile_pool(name="res", bufs=1))
        k_res = res.tile([128, OC, N], F32R, tag="k")
        q_res = res.tile([128, OC, N], F32R, tag="q")
        vT = res.tile([128, JC, E], BF16, tag="vT")
        vs_all = res.tile([128, OC, 2, H, TX], BF16, tag="vs")
        bias_t = {cn: res.tile([128, OC], F32, tag=f"b{cn}", name=f"bias_{cn}")
                  for cn in "qkv"}
        ones_b = res.tile([128, 128], BF16, tag="om")
        negC = res.tile([128, 1], F32, tag="negc")
        ident_b = res.tile([128, 128], BF16, tag="id")
        # minimax scratch lives in the persistent pool: a short-lived pool
        # makes the next pool's tiles WAR-depend on the slow gpsimd readers,
        # stalling the DVE queue (and then the tensor engine)
        m1 = res.tile([128, 1], F32, tag="m1")
        gmax = res.tile([128, 1], F32, tag="gmax")

        # ---------------- conv phase (1D winograd on x) ----------------
        with tc.tile_pool(name="vpool", bufs=1) as vpl, \
             tc.tile_pool(name="w", bufs=2) as wp, \
             tc.tile_pool(name="sc", bufs=2) as scp, \
             tc.tile_pool(name="cps", bufs=6, space="PSUM") as cps:
            V = vpl.tile([128, CC, 4, 50, TX], BF16, tag="V")

            # DMA order tuned for earliest first matmul: the u=0 rows the
            # first row tile needs, then the first weight tile, then the rest
            nc.sync.dma_start(out=V[:, :, 0, 0:19, :], in_=v_ap[:, :, 0, 0:19, :])
            w_first = wp.tile([128, CC, 3, 4, 128], BF16, tag="w")
            nc.sync.dma_start(out=w_first, in_=w_aps["k"][0])
            for u in range(1, 4):
                nc.sync.dma_start(out=V[:, :, u, 0:19, :],
                                  in_=v_ap[:, :, u, 0:19, :])
            for u in range(4):
                nc.sync.dma_start(out=V[:, :, u, 19:50, :],
                                  in_=v_ap[:, :, u, 19:50, :])
            for cn in "qkv":
                nc.sync.dma_start(out=bias_t[cn], in_=b_aps[cn])

            with tc.tile_pool(name="xw", bufs=1) as xwp:
                ident_raw = xwp.tile([128, 128], F32, tag="idr")
                make_identity(nc, ident_raw)
                nc.vector.tensor_copy(out=ident_b, in_=ident_raw)
                ones_raw = xwp.tile([128, 128], F32, tag="onr")
                nc.vector.memset(ones_raw, 1.0)
                nc.vector.tensor_copy(out=ones_b, in_=ones_raw)

            def conv(cn, sink, w0=None):
                for oc in range(OC):
                    if oc == 0 and w0 is not None:
                        w_t = w0
                    else:
                        w_t = wp.tile([128, CC, 3, 4, 128], BF16, tag="w")
                        nc.sync.dma_start(out=w_t, in_=w_aps[cn][oc])
                    for (y0, rr) in RT:
                        Ms = []
                        for u in range(4):
                            ps = cps.tile([128, rr, TX], F32, tag="m")
                            first = True
                            for cc in range(CC):
                                for ky in range(3):
                                    nc.tensor.matmul(
                                        ps, w_t[:, cc, ky, u, :],
                                        V[:, cc, u, y0 + ky:y0 + ky + rr, :],
                                        start=first,
                                        stop=(cc == CC - 1 and ky == 2),
                                    )
                                    first = False
                            Ms.append(ps)
                        sink(cn, oc, y0, rr, Ms)

            A = mybir.AluOpType

            def recombine(dv, y0, rr, Ms):
                # y_even = M0+M1+M2 ; y_odd = M1-M2-M3
                # dv is [128, 2(parity), H, TX]; pixel order is parity-major
                # (host un-permutes the final output).  tensor_tensor allows at
                # most one PSUM operand, so M1 is staged through SBUF by the
                # scalar engine.
                with nc.allow_low_precision(reason="wino recombine"):
                    a1 = scp.tile([128, rr, TX], F32, tag="a1")
                    nc.scalar.activation(
                        out=a1, in_=Ms[1],
                        func=mybir.ActivationFunctionType.Identity)
                    t0 = scp.tile([128, rr, TX], F32, tag="t0")
                    nc.vector.tensor_tensor(t0, a1, Ms[0], A.add)
                    nc.vector.tensor_tensor(
                        dv[:, 0, y0:y0 + rr, :], t0, Ms[2], A.add)
                    t1 = scp.tile([128, rr, TX], F32, tag="t1")
                    nc.vector.tensor_tensor(t1, a1, Ms[2], A.subtract)
                    nc.vector.tensor_tensor(
                        dv[:, 1, y0:y0 + rr, :], t1, Ms[3], A.subtract)

            def to_res(dst):
                dvw = dst.rearrange("p o (two y x) -> p o two y x",
                                    y=H, x=TX, two=2)

                def sink(cn, oc, y0, rr, Ms):
                    recombine(dvw[:, oc], y0, rr, Ms)
                    if y0 + rr == H:  # last row tile: apply bias to whole oc slice
                        nc.scalar.activation(
                            out=dst[:, oc, :], in_=dst[:, oc, :],
                            func=mybir.ActivationFunctionType.Identity,
                            bias=bias_t[cn][:, oc:oc + 1], scale=1.0,
                        )
                return sink

            conv("k", to_res(k_res), w0=w_first)
            conv("q", to_res(q_res))

            # ---- global shift constant C (sampled block of s) ----
            # softmax is shift-invariant; any C with rowmax-80 <= C <= gmax+88
            # keeps exp() in fp32 range, so a sampled-block max works.
            with tc.tile_pool(name="mps", bufs=1, space="PSUM") as mps:
                mini = mps.tile([128, 2, 256], F32)
                for jc in range(2):
                    for ec in range(OC):
                        nc.tensor.matmul(
                            mini[:, jc, :], k_res[:, ec, jc * 128:(jc + 1) * 128],
                            q_res[:, ec, 0:256], start=(ec == 0), stop=(ec == OC - 1),
                        )
                nc.vector.reduce_max(out=m1, in_=mini, axis=mybir.AxisListType.XY)
                nc.gpsimd.partition_all_reduce(
                    gmax, m1, channels=128, reduce_op=bass_isa.ReduceOp.max)
                # negate on gpsimd: keeps the slow cross-engine round-trip off
                # the in-order DVE queue (only the scalar-engine exp reads negC)
                nc.gpsimd.tensor_scalar_mul(negC, gmax, -1.0)

            # v conv: recombine into resident bf16 vs, transpose into vT
            with tc.tile_pool(name="tps", bufs=2, space="PSUM") as tps:
                def v_sink(cn, oc, y0, rr, Ms):
                    recombine(vs_all[:, oc], y0, rr, Ms)
                    if y0 + rr == H:
                        vsf = vs_all.rearrange("p o two y x -> p o (two y x)")[:, oc]
                        nc.scalar.activation(
                            out=vsf, in_=vsf,
                            func=mybir.ActivationFunctionType.Identity,
                            bias=bias_t["v"][:, oc:oc + 1], scale=1.0,
                        )
                        for jc in range(JC):
                            tp = tps.tile([128, 128], BF16, tag="t")
                            nc.tensor.transpose(
                                tp, vsf[:, jc * 128:(jc + 1) * 128], ident_b)
                            nc.scalar.activation(
                                out=vT[:, jc, oc * 128:(oc + 1) * 128], in_=tp,
                                func=mybir.ActivationFunctionType.Identity)

                conv("v", v_sink)

        # ---------------- attention ----------------
        vs_flat = vs_all.rearrange("p o two y x -> p o (two y x)")
        with tc.tile_pool(name="pp", bufs=2) as pp, \
             tc.tile_pool(name="esb", bufs=2) as esb, \
             tc.tile_pool(name="sps", bufs=3, space="PSUM") as sps, \
             tc.tile_pool(name="aps", bufs=4, space="PSUM") as aps, \
             tc.tile_pool(name="rps", bufs=1, space="PSUM") as rps:
            p_tiles = {}

            def emit_qk(t):
                i0, iw = IT[t]
                p_t = pp.tile([128, JC, iw], BF16, tag="p")
                p_tiles[t] = p_t
                for jc in range(JC):
                    ps = sps.tile([128, iw], F32, tag="s")
                    for ec in range(OC):
                        nc.tensor.matmul(
                            ps, k_res[:, ec, jc * 128:(jc + 1) * 128],
                            q_res[:, ec, i0:i0 + iw],
                            start=(ec == 0), stop=(ec == OC - 1),
                        )
                    nc.scalar.activation(
                        out=p_t[:, jc, :], in_=ps,
                        func=mybir.ActivationFunctionType.Exp,
                        bias=negC[:, 0:1], scale=1.0,
                    )

            def emit_post(t):
                i0, iw = IT[t]
                p_t = p_tiles.pop(t)
                # row sums, pre-broadcast to all 128 partitions via ones matrix
                rs = rps.tile([128, iw], F32, tag="rs")
                for jc in range(JC):
                    nc.tensor.matmul(rs, ones_b, p_t[:, jc, :],
                                     start=(jc == 0), stop=(jc == JC - 1))
                r_inv = esb.tile([128, iw], F32, tag="r")
                nc.vector.reciprocal_approx_fast(out=r_inv, in_=rs)
                avs = []
                for ec in range(OC):
                    av = aps.tile([128, iw], F32, tag="av", name=f"av_{t}_{ec}")
                    for jc in range(JC):
                        nc.tensor.matmul(
                            av, vT[:, jc, ec * 128:(ec + 1) * 128], p_t[:, jc, :],
                            start=(jc == 0), stop=(jc == JC - 1),
                        )
                    avs.append(av)
                for ec in range(OC):
                    o_t = esb.tile([128, iw], F32, tag="o", name=f"o_{t}_{ec}")
                    nc.vector.tensor_tensor(o_t, avs[ec], r_inv,
                                            mybir.AluOpType.mult)
                    nc.vector.tensor_tensor(o_t, o_t, vs_flat[:, ec, i0:i0 + iw],
                                            mybir.AluOpType.add)
                    nc.sync.dma_start(out=out_ap[ec, :, i0:i0 + iw], in_=o_t)

            emit_qk(0)
            for t in range(1, len(IT)):
                emit_qk(t)
                emit_post(t - 1)
            emit_post(len(IT) - 1)

    nc.compile()
    return nc


def _prep_shared(Wq, bq, Wk, bk, Wv, bv):
    def wprep(Wm):
        Am = Wm.reshape(OC, 128, CC, 128, 3, 3).astype(np.float64)
        w0, w1, w2 = Am[..., 0], Am[..., 1], Am[..., 2]   # [oc, o, cc, c, ky]
        U = np.stack([w0, (w0 + w1 + w2) * 0.5, (w0 - w1 + w2) * 0.5, w2],
                     axis=-1)                              # [oc, o, cc, c, ky, u]
        U = U.transpose(0, 3, 2, 4, 5, 1)                  # [oc, c, cc, ky, u, o]
        return np.ascontiguousarray(
            U.reshape(OC, 128, CC, 3, 4, 128).astype(ml_dtypes.bfloat16))

    def bprep(bm):
        return np.ascontiguousarray(bm.reshape(OC, 128).T, dtype=np.float32)

    return {
        "wq": wprep(Wq), "wk": wprep(Wk), "wv": wprep(Wv),
        "bq": bprep(bq), "bk": bprep(bk), "bv": bprep(bv),
    }


def kernel(feat, Wq, bq, Wk, bk, Wv, bv):
    feat = np.asarray(feat, dtype=np.float32)
    if "nc" not in _CACHE:
        _CACHE["nc"] = _build()
    nc = _CACHE["nc"]

    shared = _prep_shared(np.asarray(Wq, np.float32), np.asarray(bq, np.float32),
                          np.asarray(Wk, np.float32), np.asarray(bk, np.float32),
                          np.asarray(Wv, np.float32), np.asarray(bv, np.float32))

    in_maps = []
    for b in range(B):
        xpad = np.zeros((C, 50, 50), np.float32)
        xpad[:, 1:49, 1:49] = feat[b]
        # host-side winograd input transform B^T d over stride-2 x tiles
        d0 = xpad[:, :, 0:48:2]
        d1 = xpad[:, :, 1:48:2]
        d2 = xpad[:, :, 2:50:2]
        d3 = xpad[:, :, 3:50:2]
        Vh = np.stack([d0 - d2, d1 + d2, d2 - d1, d1 - d3], axis=1)  # [C,4,50,24]
        Vh = np.ascontiguousarray(
            Vh.reshape(CC, 128, 4, 50, TX).transpose(1, 0, 2, 3, 4)
            .astype(ml_dtypes.bfloat16))
        in_maps.append({"vwino": Vh, **shared})

    r = bass_utils.run_bass_kernel_spmd(nc, in_maps, list(range(B)))
    # device pixel order is parity-major [2, H, TX]; un-permute to [H, W]
    out = np.stack(
        [r.results[b]["out"].reshape(E, 2, H, TX).transpose(0, 2, 3, 1)
         .reshape(E, H, W) for b in range(B)], axis=0
    )
    return out
